# revision 6
# baseline (speedup 1.0000x reference)
"""CenterLoss kernel for Trainium2 (8 NeuronCores, SPMD data-parallel).

loss = sum(clip(distmat * onehot(labels), 1e-12, 1e12)) / B
with distmat[b,c] = ||x_b - centers_c||^2.

Only distmat[b, labels[b]] survives the mask; every other (B*(C-1)) entry is
a masked zero that the clip turns into exactly 1e-12. So the kernel gathers
centers[labels[b]] per row (indirect DMA), computes the per-row squared
distance, clamps at 1e-12, and the host adds the closed-form constant
B*(C-1)*1e-12 before dividing by B.

Sharding: batch split 8 ways (1024 rows/core), centers replicated.

HW notes (found the hard way on this runtime):
- In-place DVE ops (out aliasing an input) and the custom
  tensor_tensor_reduce ISA op both kill the exec unit
  (NRT_EXEC_UNIT_UNRECOVERABLE); use fresh output tiles and
  ACT Square+accum_out for the row reduction instead.
- Use Bacc (not Bass): its finalize() splits multi-sem waits, which TRN2
  codegen rejects (one sync-wait per instruction), and bass2jax serializes
  the module without finalizing, so finalize before running.
"""

import numpy as np

from concourse import bacc, bass, mybir
import concourse.tile as tile
from concourse.bass_utils import run_bass_kernel_spmd

B = 8192
C = 10000
D = 256
N_CORES = 8
BL = B // N_CORES  # rows per core
P = 128            # SBUF partitions
T = BL // P        # row-tiles per core

_CLIP_LO = 1e-12

_nc_cache = None


def _build():
    global _nc_cache
    if _nc_cache is not None:
        return _nc_cache

    nc = bacc.Bacc()
    x_l = nc.dram_tensor("x_local", [BL, D], mybir.dt.float32, kind="ExternalInput")
    lab_l = nc.dram_tensor("labels_local", [BL], mybir.dt.int32, kind="ExternalInput")
    cen = nc.dram_tensor("centers", [C, D], mybir.dt.float32, kind="ExternalInput")
    out = nc.dram_tensor("partials", [P, T], mybir.dt.float32, kind="ExternalOutput")

    with tile.TileContext(nc) as tc:
        with (
            tc.tile_pool(name="sbuf", bufs=3) as sbuf,
            tc.tile_pool(name="accp", bufs=1) as accp,
        ):
            acc = accp.tile([P, T], mybir.dt.float32)
            for t in range(T):
                xt = sbuf.tile([P, D], mybir.dt.float32)
                ct = sbuf.tile([P, D], mybir.dt.float32)
                dt = sbuf.tile([P, D], mybir.dt.float32)
                sq = sbuf.tile([P, D], mybir.dt.float32)
                lt = sbuf.tile([P, 1], mybir.dt.int32)
                nc.sync.dma_start(out=xt[:], in_=x_l[t * P:(t + 1) * P, :])
                nc.sync.dma_start(out=lt[:], in_=lab_l[t * P:(t + 1) * P, None])
                nc.gpsimd.indirect_dma_start(
                    out=ct[:],
                    out_offset=None,
                    in_=cen[:],
                    in_offset=bass.IndirectOffsetOnAxis(ap=lt[:, :1], axis=0),
                )
                nc.vector.tensor_sub(out=dt[:], in0=xt[:], in1=ct[:])
                nc.scalar.activation(
                    out=sq[:],
                    in_=dt[:],
                    func=mybir.ActivationFunctionType.Square,
                    accum_out=acc[:, t:t + 1],
                )
            acc2 = accp.tile([P, T], mybir.dt.float32)
            nc.vector.tensor_scalar_max(out=acc2[:], in0=acc[:], scalar1=_CLIP_LO)
            nc.sync.dma_start(out=out[:], in_=acc2[:])

    nc.finalize()
    _nc_cache = nc
    return nc


def _run(x, labels, centers, **spmd_kwargs):
    nc = _build()
    x = np.ascontiguousarray(np.asarray(x), dtype=np.float32)
    labels = np.ascontiguousarray(np.asarray(labels)).astype(np.int32)
    centers = np.ascontiguousarray(np.asarray(centers), dtype=np.float32)

    in_maps = []
    for c in range(N_CORES):
        sl = slice(c * BL, (c + 1) * BL)
        in_maps.append(
            {
                "x_local": x[sl],
                "labels_local": labels[sl],
                "centers": centers,
            }
        )
    res = run_bass_kernel_spmd(nc, in_maps, list(range(N_CORES)), **spmd_kwargs)
    partials = np.stack([r["partials"] for r in res.results])  # [8, P, T]
    total = partials.astype(np.float64).sum()
    loss = (total + B * (C - 1) * _CLIP_LO) / B
    return np.asarray(loss, dtype=np.float32), res


def kernel(x, labels, centers):
    loss, _ = _run(x, labels, centers)
    return loss
